# revision 2
# baseline (speedup 1.0000x reference)
"""Multi-head attention (RoPE, causal) on 8 TRN2 NeuronCores.

Sharding: core c -> batch b = c//2, head-group g = c%2 (8 of 16 heads).

v5 design:
- all attention operands bf16 (PSUM accumulation stays f32); everything
  SBUF-resident, no DRAM staging roundtrips.
- q/k stored per head-PAIR: head A dims on partitions 0-63, head B on
  64-127, natural dim order (rope pairs adjacent). The rope cross-term
  partition swap is a stream_shuffle with mask i^1; the sin sign pattern
  is baked into SN host-side. Scores for the two heads issue as two K=64
  matmuls with tile_position (0,0)/(64,0) into adjacent PSUM banks ->
  concurrent on HW.
- exp reads the pair's [128, 1024] PSUM span in ONE activation.
- V' carries 64 replicated ones-columns (M=128): pO rows 64-127 hold the
  softmax denominator replicated 64x, so normalize is reciprocal+mul
  with no partition broadcast. Matmul cost is free-dim based, so the
  extra 63 columns are free.
- query chunks of 512: causal work only; o_proj per chunk overlaps the
  last pair's attention.
"""

import numpy as np
from contextlib import ExitStack

import ml_dtypes

import concourse.bacc as bacc
import concourse.bass as bass
import concourse.mybir as mybir
import concourse.tile as tile
from concourse.bass_utils import run_bass_kernel_spmd
from concourse.masks import make_upper_triangular

F32 = mybir.dt.float32
BF16 = mybir.dt.bfloat16
AF = mybir.ActivationFunctionType

D = 1024
S = 2048
NH = 16
DK = 64
HPC = 8          # heads per core
HD = HPC * DK    # 512
NCORES = 8
THETA = 10000.0

NK = D // 128    # 8 x-dim k-tiles
NS = S // 128    # 16 key tiles
NC_ = 4          # query chunks of 512
NP = 4           # head pairs per core

_CACHE = {}

_SWAP_MASK = [i ^ 1 for i in range(32)]


def _copy(nc, use_scalar, out, in_):
    if use_scalar:
        nc.scalar.copy(out, in_)
    else:
        nc.vector.tensor_copy(out, in_)


def _build_nc():
    nc = bacc.Bacc(None, target_bir_lowering=False)

    XT = nc.dram_tensor("XT", [128, NC_, NK, 512], BF16, kind="ExternalInput")
    WQ = nc.dram_tensor("WQ", [128, NK, HD], BF16, kind="ExternalInput")
    WK = nc.dram_tensor("WK", [128, NK, HD], BF16, kind="ExternalInput")
    WV = nc.dram_tensor("WV", [128, NK, HD], BF16, kind="ExternalInput")
    OC = nc.dram_tensor("OC", [128, HD // 128, D], BF16, kind="ExternalInput")
    CC = nc.dram_tensor("CC", [128, S], F32, kind="ExternalInput")
    SN = nc.dram_tensor("SN", [128, S], F32, kind="ExternalInput")
    OT = nc.dram_tensor("OT", [D, S], F32, kind="ExternalOutput")

    with tile.TileContext(nc) as tc, ExitStack() as ctx:
        const = ctx.enter_context(tc.tile_pool(name="const", bufs=1))
        resv = ctx.enter_context(tc.tile_pool(name="resv", bufs=1))
        psp = ctx.enter_context(tc.tile_pool(name="psp", bufs=2,
                                             space="PSUM"))
        rp = ctx.enter_context(tc.tile_pool(name="rp", bufs=2))
        ptp = ctx.enter_context(tc.tile_pool(name="ptp", bufs=3))
        rp2 = ctx.enter_context(tc.tile_pool(name="rp2", bufs=2))

        # constants
        tri32 = const.tile([128, 128], F32, tag="tri32")
        make_upper_triangular(nc, tri32, val=1.0, diag=True)
        tri = const.tile([128, 128], BF16, tag="tri")
        nc.vector.tensor_copy(tri, tri32)
        cc_sb = const.tile([128, S], F32, tag="cc")
        sn_sb = const.tile([128, S], F32, tag="sn")

        # resident tensors
        xtc = [resv.tile([128, NK, 512], BF16, tag=f"xtc{c}",
                         name=f"xtc{c}") for c in range(NC_)]
        vp = resv.tile([128, NS, HPC * 128], BF16, tag="vp")
        wq_sb = resv.tile([128, NK, HD], BF16, tag="wq")
        wk_sb = resv.tile([128, NK, HD], BF16, tag="wk")
        wv_sb = resv.tile([128, NK, HD], BF16, tag="wv")
        oc_sb = resv.tile([128, HD // 128, D], BF16, tag="oc")
        qp = [resv.tile([128, S], BF16, tag=f"qp{p}", name=f"qp{p}")
              for p in range(NP)]
        kp = [resv.tile([128, S], BF16, tag=f"kp{p}", name=f"kp{p}")
              for p in range(NP)]
        ypr = [resv.tile([128, S], BF16, tag=f"y{p}", name=f"y{p}")
               for p in range(NP)]

        # input DMAs: xtall by s-chunk so chunk-0 projections start early;
        # spread loads over the three DMA-capable queues by criticality
        def xt_chunk(eng, cd):
            eng.dma_start(out=xtc[cd], in_=XT[:, cd])

        xt_chunk(nc.sync, 0)
        nc.scalar.dma_start(out=wq_sb, in_=WQ[:, :, :])
        nc.gpsimd.dma_start(out=cc_sb, in_=CC[:, :])
        nc.sync.dma_start(out=wk_sb, in_=WK[:, :, :])
        nc.scalar.dma_start(out=sn_sb, in_=SN[:, :])
        xt_chunk(nc.gpsimd, 1)
        xt_chunk(nc.sync, 2)
        xt_chunk(nc.scalar, 3)
        nc.sync.dma_start(out=wv_sb, in_=WV[:, :, :])
        nc.scalar.dma_start(out=oc_sb, in_=OC[:, :, :])

        def emit_proj_chunk(wsb, p, c, dest):
            """q/k projection + rope for pair p, query chunk c."""
            cs = slice(c * 512, (c + 1) * 512)
            ps = psp.tile([128, 512], F32, tag="ps", name="ps_proj")
            for k in range(NK):
                nc.tensor.matmul(ps, wsb[:, k, p * 128:(p + 1) * 128],
                                 xtc[c][:, k, :],
                                 start=(k == 0), stop=(k == NK - 1))
            u = rp.tile([128, 512], BF16, tag="U", name="u")
            nc.vector.tensor_mul(u, ps, cc_sb[:, cs])
            w2 = rp.tile([128, 512], BF16, tag="W2", name="w2")
            nc.vector.tensor_mul(w2, ps, sn_sb[:, cs])
            # rope combine: out[2i] = e*c - o*s ; out[2i+1] = o*c + e*s
            # (sign baked into SN; swap adjacent partitions then add)
            w2s = rp.tile([128, 512], BF16, tag="W2s", name="w2s")
            nc.vector.stream_shuffle(w2s, w2, _SWAP_MASK)
            nc.gpsimd.tensor_add(dest[:, cs], u, w2s)

        def emit_v_tile(j):
            # ones columns of V' (denominator trick). Head A blocks are
            # [dims | ones]; head B blocks are [ones | dims] so B's
            # numerator lands on partitions 64-127 (same-start muls).
            vj0 = vp[:, j, :].rearrange("p (q two e) -> p q two e",
                                        two=2, e=128)
            nc.gpsimd.memset(vj0[:, :, 0, DK:128], 1.0)
            nc.gpsimd.memset(vj0[:, :, 1, 0:DK], 1.0)
            psv = psp.tile([128, 512], F32, tag="ps", name="psv")
            for k in range(NK):
                nc.tensor.matmul(psv,
                                 xtc[j // 4][:, k, (j % 4) * 128:(j % 4 + 1) * 128],
                                 wv_sb[:, k, :],
                                 start=(k == 0), stop=(k == NK - 1))
            vj = vp[:, j, :].rearrange("p (q two e) -> p q two e",
                                       two=2, e=128)
            sv = psv.rearrange("p (q two e) -> p q two e", two=2, e=DK)
            _copy(nc, j % 2 == 0, vj[:, :, 0, 0:DK], sv[:, :, 0, :])
            _copy(nc, j % 2 == 1, vj[:, :, 1, DK:128], sv[:, :, 1, :])

        def emit_attn_chunk(p, c):
            cs = slice(c * 512, (c + 1) * 512)
            jmax = 4 * c + 3
            pOp = psp.tile([128, 1024], F32, tag="pO", bufs=1, name="pOp")
            for j in range(jmax + 1):
                off = j * 128
                d = j - 4 * c
                nlo = max(0, 128 * d)
                qv = slice(c * 512 + nlo, (c + 1) * 512)
                pS = psp.tile([128, 1024], F32, tag="pS", bufs=2, name="pS")
                nc.tensor.matmul(pS[:, nlo:512],
                                 kp[p][0:64, off:off + 128],
                                 qp[p][0:64, qv],
                                 start=True, stop=True,
                                 tile_position=(0, 0))
                nc.tensor.matmul(pS[:, 512 + nlo:1024],
                                 kp[p][64:128, off:off + 128],
                                 qp[p][64:128, qv],
                                 start=True, stop=True,
                                 tile_position=(64, 0))
                pt = ptp.tile([128, 1024], BF16, tag="pt", name="pt")
                if d < 0:
                    nc.scalar.activation(pt, pS, AF.Exp, scale=0.125)
                else:
                    pt_r = pt.rearrange("p (h w) -> p h w", w=512)[:, :, nlo:]
                    pS_r = pS.rearrange("p (h w) -> p h w", w=512)[:, :, nlo:]
                    nc.scalar.activation(pt_r, pS_r, AF.Exp, scale=0.125)
                    for hh in range(2):
                        blk = slice(512 * hh + nlo, 512 * hh + nlo + 128)
                        nc.gpsimd.tensor_mul(pt[:, blk], pt[:, blk], tri)
                for hh in range(2):
                    vsl = vp[:, j, 128 * (2 * p + hh):128 * (2 * p + hh + 1)]
                    nc.tensor.matmul(
                        pOp[:, 512 * hh + nlo:512 * (hh + 1)],
                        vsl,
                        pt[:, 512 * hh + nlo:512 * (hh + 1)],
                        start=(j == 0), stop=(j == jmax))
            # pO layout: A num rows 0-63 / den rows 64-127 (cols 0-511);
            # B den rows 0-63 / num rows 64-127 (cols 512-1023)
            osb = rp2.tile([128, 1024], F32, tag="osb", name="osb")
            nc.vector.tensor_copy(osb, pOp)
            recb = rp2.tile([128, 1024], F32, tag="recb", name="recb")
            nc.vector.reciprocal(recb[0:64, 0:512], osb[64:128, 0:512])
            nc.vector.reciprocal(recb[64:128, 512:1024],
                                 osb[0:64, 512:1024])
            nc.vector.tensor_mul(ypr[p][0:64, cs], osb[0:64, 0:512],
                                 recb[0:64, 0:512])
            nc.vector.tensor_mul(ypr[p][64:128, cs],
                                 osb[64:128, 512:1024],
                                 recb[64:128, 512:1024])

        def emit_oproj_chunk(c):
            cs = slice(c * 512, (c + 1) * 512)
            for dt in range(8):
                po = psp.tile([128, 512], F32, tag="ps", name="po")
                for kk in range(4):
                    nc.tensor.matmul(po,
                                     oc_sb[:, kk, dt * 128:(dt + 1) * 128],
                                     ypr[kk][:, cs],
                                     start=(kk == 0), stop=(kk == 3))
                osb2 = rp2.tile([128, 512], F32, tag="os", bufs=3,
                                name="osb2")
                _copy(nc, (dt + c) % 2 == 0, osb2, po)
                nc.gpsimd.dma_start(out=OT[dt * 128:(dt + 1) * 128, cs],
                                    in_=osb2)

        # emission: pair-0 projections, then pair-outer attention with
        # next-pair projections and V tiles interleaved
        for c in range(NC_):
            emit_proj_chunk(wq_sb, 0, c, qp[0])
        for c in range(NC_):
            emit_proj_chunk(wk_sb, 0, c, kp[0])
        for j in range(4):
            emit_v_tile(j)
        for p in range(NP):
            for c in range(NC_):
                emit_attn_chunk(p, c)
                if p == 0 and c < 3:
                    for j in range(4 * c + 4, 4 * c + 8):
                        emit_v_tile(j)
                if p < 3:
                    emit_proj_chunk(wq_sb, p + 1, c, qp[p + 1])
                    emit_proj_chunk(wk_sb, p + 1, c, kp[p + 1])
        # o_proj emitted last: lowest priority, so its matmuls fill PE
        # gaps while the final pair's exp-bound attention drains
        for c in range(NC_):
            emit_oproj_chunk(c)

    nc.finalize()
    return nc


def _prep_inputs(x, q_proj, k_proj, v_proj, o_proj):
    bf16 = ml_dtypes.bfloat16
    pos = np.arange(S, dtype=np.float64)
    inv = THETA ** (-np.arange(0, DK, 2, dtype=np.float64) / DK)   # [32]
    ang = inv[:, None] * pos[None, :]                              # [32, S]
    cos32, sin32 = np.cos(ang), np.sin(ang)
    # interleaved rope rows: row 2i -> freq i; sin sign: + on even rows
    # (multiplies e_i for the odd output), - on odd rows
    cos64 = np.repeat(cos32, 2, axis=0)                            # [64, S]
    sin64 = np.repeat(sin32, 2, axis=0)
    sin64[1::2] *= -1.0
    cos_big = np.tile(cos64, (2, 1)).astype(np.float32)            # [128, S]
    sin_big = np.tile(sin64, (2, 1)).astype(np.float32)

    in_maps = []
    for core in range(NCORES):
        b, g = core // 2, core % 2
        heads = [g * HPC + i for i in range(HPC)]
        nat = [h * DK + d_ for h in heads for d_ in range(DK)]

        def wlay(w):   # [D, HD] -> [128, NK, HD]
            return np.ascontiguousarray(
                w.reshape(NK, 128, HD).transpose(1, 0, 2)).astype(bf16)

        xt4 = x[b].T.reshape(NK, 128, NC_, 512).transpose(1, 2, 0, 3)
        oc4 = o_proj[:, nat].T.reshape(HD // 128, 128, D).transpose(1, 0, 2)
        in_maps.append({
            "XT": np.ascontiguousarray(xt4).astype(bf16),
            "WQ": wlay(q_proj[nat, :].T),
            "WK": wlay(k_proj[nat, :].T),
            "WV": wlay(v_proj[nat, :].T),
            "OC": np.ascontiguousarray(oc4).astype(bf16),
            "CC": cos_big,
            "SN": sin_big,
        })
    return in_maps


def _run(in_maps, **kw):
    if "nc" not in _CACHE:
        _CACHE["nc"] = _build_nc()
    return run_bass_kernel_spmd(_CACHE["nc"], in_maps,
                                core_ids=list(range(NCORES)), **kw)


def kernel(x, q_proj, k_proj, v_proj, o_proj):
    x = np.asarray(x, dtype=np.float32)
    in_maps = _prep_inputs(x,
                           np.asarray(q_proj, dtype=np.float32),
                           np.asarray(k_proj, dtype=np.float32),
                           np.asarray(v_proj, dtype=np.float32),
                           np.asarray(o_proj, dtype=np.float32))
    res = _run(in_maps)
    B = x.shape[0]
    out = np.empty((B, S, D), dtype=np.float32)
    for b in range(B):
        ot = res.results[2 * b]["OT"] + res.results[2 * b + 1]["OT"]
        out[b] = ot.T
    return out


# revision 3
# speedup vs baseline: 1.0936x; 1.0936x over previous
"""Multi-head attention (RoPE, causal) on 8 TRN2 NeuronCores.

Sharding: core c -> batch b = c//2, head-group g = c%2 (8 of 16 heads).

v5 design:
- all attention operands bf16 (PSUM accumulation stays f32); everything
  SBUF-resident, no DRAM staging roundtrips.
- q/k stored per head-PAIR: head A dims on partitions 0-63, head B on
  64-127, natural dim order (rope pairs adjacent). The rope cross-term
  partition swap is a stream_shuffle with mask i^1; the sin sign pattern
  is baked into SN host-side. Scores for the two heads issue as two K=64
  matmuls with tile_position (0,0)/(64,0) into adjacent PSUM banks ->
  concurrent on HW.
- exp reads the pair's [128, 1024] PSUM span in ONE activation.
- V' carries 64 replicated ones-columns (M=128): pO rows 64-127 hold the
  softmax denominator replicated 64x, so normalize is reciprocal+mul
  with no partition broadcast. Matmul cost is free-dim based, so the
  extra 63 columns are free.
- query chunks of 512: causal work only; o_proj per chunk overlaps the
  last pair's attention.
"""

import numpy as np
from contextlib import ExitStack

import ml_dtypes

import concourse.bacc as bacc
import concourse.bass as bass
import concourse.mybir as mybir
import concourse.tile as tile
from concourse.bass_utils import run_bass_kernel_spmd
from concourse.masks import make_upper_triangular

F32 = mybir.dt.float32
BF16 = mybir.dt.bfloat16
AF = mybir.ActivationFunctionType

D = 1024
S = 2048
NH = 16
DK = 64
HPC = 8          # heads per core
HD = HPC * DK    # 512
NCORES = 8
THETA = 10000.0

NK = D // 128    # 8 x-dim k-tiles
NS = S // 128    # 16 key tiles
NC_ = 4          # query chunks of 512
NP = 4           # head pairs per core

_CACHE = {}

_SWAP_MASK = [i ^ 1 for i in range(32)]


def _copy(nc, use_scalar, out, in_):
    if use_scalar:
        nc.scalar.copy(out, in_)
    else:
        nc.vector.tensor_copy(out, in_)


def _build_nc():
    nc = bacc.Bacc(None, target_bir_lowering=False)

    XT = nc.dram_tensor("XT", [128, NC_, NK, 512], BF16, kind="ExternalInput")
    WQ = nc.dram_tensor("WQ", [128, NK, HD], BF16, kind="ExternalInput")
    WK = nc.dram_tensor("WK", [128, NK, HD], BF16, kind="ExternalInput")
    WV = nc.dram_tensor("WV", [128, NK, HD], BF16, kind="ExternalInput")
    OC = nc.dram_tensor("OC", [128, HD // 128, D], BF16, kind="ExternalInput")
    CC = nc.dram_tensor("CC", [128, S], F32, kind="ExternalInput")
    SN = nc.dram_tensor("SN", [128, S], F32, kind="ExternalInput")
    OT = nc.dram_tensor("OT", [D, S], F32, kind="ExternalOutput")

    with tile.TileContext(nc) as tc, ExitStack() as ctx:
        const = ctx.enter_context(tc.tile_pool(name="const", bufs=1))
        resv = ctx.enter_context(tc.tile_pool(name="resv", bufs=1))
        psp = ctx.enter_context(tc.tile_pool(name="psp", bufs=2,
                                             space="PSUM"))
        rp = ctx.enter_context(tc.tile_pool(name="rp", bufs=2))
        ptp = ctx.enter_context(tc.tile_pool(name="ptp", bufs=3))
        rp2 = ctx.enter_context(tc.tile_pool(name="rp2", bufs=2))

        # constants
        tri32 = const.tile([128, 128], F32, tag="tri32")
        make_upper_triangular(nc, tri32, val=1.0, diag=True)
        tri = const.tile([128, 128], BF16, tag="tri")
        nc.vector.tensor_copy(tri, tri32)
        cc_sb = const.tile([128, S], F32, tag="cc")
        sn_sb = const.tile([128, S], F32, tag="sn")

        # resident tensors
        xtc = [resv.tile([128, NK, 512], BF16, tag=f"xtc{c}",
                         name=f"xtc{c}") for c in range(NC_)]
        vp = resv.tile([128, NS, HPC * 128], BF16, tag="vp")
        wq_sb = resv.tile([128, NK, HD], BF16, tag="wq")
        wk_sb = resv.tile([128, NK, HD], BF16, tag="wk")
        wv_sb = resv.tile([128, NK, HD], BF16, tag="wv")
        oc_sb = resv.tile([128, HD // 128, D], BF16, tag="oc")
        qp = [resv.tile([128, S], BF16, tag=f"qp{p}", name=f"qp{p}")
              for p in range(NP)]
        kp = [resv.tile([128, S], BF16, tag=f"kp{p}", name=f"kp{p}")
              for p in range(NP)]
        ypr = [resv.tile([128, S], BF16, tag=f"y{p}", name=f"y{p}")
               for p in range(NP)]

        # input DMAs: xtall by s-chunk so chunk-0 projections start early;
        # spread loads over the three DMA-capable queues by criticality
        def xt_chunk(eng, cd):
            eng.dma_start(out=xtc[cd], in_=XT[:, cd])

        xt_chunk(nc.sync, 0)
        nc.scalar.dma_start(out=wq_sb, in_=WQ[:, :, :])
        nc.gpsimd.dma_start(out=cc_sb, in_=CC[:, :])
        nc.sync.dma_start(out=wk_sb, in_=WK[:, :, :])
        nc.scalar.dma_start(out=sn_sb, in_=SN[:, :])
        xt_chunk(nc.gpsimd, 1)
        xt_chunk(nc.sync, 2)
        xt_chunk(nc.scalar, 3)
        nc.sync.dma_start(out=wv_sb, in_=WV[:, :, :])
        nc.scalar.dma_start(out=oc_sb, in_=OC[:, :, :])

        def emit_proj_chunk(wsb, p, c, dest):
            """q/k projection + rope for pair p, query chunk c."""
            cs = slice(c * 512, (c + 1) * 512)
            ps = psp.tile([128, 512], F32, tag="ps", name="ps_proj")
            for k in range(NK):
                nc.tensor.matmul(ps, wsb[:, k, p * 128:(p + 1) * 128],
                                 xtc[c][:, k, :],
                                 start=(k == 0), stop=(k == NK - 1))
            u = rp.tile([128, 512], F32, tag="U", name="u")
            nc.vector.tensor_mul(u, ps, cc_sb[:, cs])
            w2 = rp.tile([128, 512], F32, tag="W2", name="w2")
            nc.vector.tensor_mul(w2, ps, sn_sb[:, cs])
            # rope combine: out[2i] = e*c - o*s ; out[2i+1] = o*c + e*s
            # (sign baked into SN; swap adjacent partitions then add).
            # f32 temporaries: only the final add rounds to bf16.
            w2s = rp.tile([128, 512], F32, tag="W2s", name="w2s")
            nc.vector.stream_shuffle(w2s, w2, _SWAP_MASK)
            nc.gpsimd.tensor_add(dest[:, cs], u, w2s)

        def emit_v_tile(j):
            # ones columns of V' (denominator trick). Head A blocks are
            # [dims | ones]; head B blocks are [ones | dims] so B's
            # numerator lands on partitions 64-127 (same-start muls).
            vj0 = vp[:, j, :].rearrange("p (q two e) -> p q two e",
                                        two=2, e=128)
            nc.gpsimd.memset(vj0[:, :, 0, DK:128], 1.0)
            nc.gpsimd.memset(vj0[:, :, 1, 0:DK], 1.0)
            psv = psp.tile([128, 512], F32, tag="ps", name="psv")
            for k in range(NK):
                nc.tensor.matmul(psv,
                                 xtc[j // 4][:, k, (j % 4) * 128:(j % 4 + 1) * 128],
                                 wv_sb[:, k, :],
                                 start=(k == 0), stop=(k == NK - 1))
            vj = vp[:, j, :].rearrange("p (q two e) -> p q two e",
                                       two=2, e=128)
            sv = psv.rearrange("p (q two e) -> p q two e", two=2, e=DK)
            _copy(nc, j % 2 == 0, vj[:, :, 0, 0:DK], sv[:, :, 0, :])
            _copy(nc, j % 2 == 1, vj[:, :, 1, DK:128], sv[:, :, 1, :])

        def emit_attn_chunk(p, c):
            cs = slice(c * 512, (c + 1) * 512)
            jmax = 4 * c + 3
            pOp = psp.tile([128, 1024], F32, tag="pO", bufs=1, name="pOp")
            for j in range(jmax + 1):
                off = j * 128
                d = j - 4 * c
                nlo = max(0, 128 * d)
                qv = slice(c * 512 + nlo, (c + 1) * 512)
                pS = psp.tile([128, 1024], F32, tag="pS", bufs=2, name="pS")
                nc.tensor.matmul(pS[:, nlo:512],
                                 kp[p][0:64, off:off + 128],
                                 qp[p][0:64, qv],
                                 start=True, stop=True,
                                 tile_position=(0, 0))
                nc.tensor.matmul(pS[:, 512 + nlo:1024],
                                 kp[p][64:128, off:off + 128],
                                 qp[p][64:128, qv],
                                 start=True, stop=True,
                                 tile_position=(64, 0))
                pt = ptp.tile([128, 1024], BF16, tag="pt", name="pt")
                if d < 0:
                    nc.scalar.activation(pt, pS, AF.Exp, scale=0.125)
                else:
                    pt_r = pt.rearrange("p (h w) -> p h w", w=512)[:, :, nlo:]
                    pS_r = pS.rearrange("p (h w) -> p h w", w=512)[:, :, nlo:]
                    nc.scalar.activation(pt_r, pS_r, AF.Exp, scale=0.125)
                    for hh in range(2):
                        blk = slice(512 * hh + nlo, 512 * hh + nlo + 128)
                        nc.gpsimd.tensor_mul(pt[:, blk], pt[:, blk], tri)
                for hh in range(2):
                    vsl = vp[:, j, 128 * (2 * p + hh):128 * (2 * p + hh + 1)]
                    nc.tensor.matmul(
                        pOp[:, 512 * hh + nlo:512 * (hh + 1)],
                        vsl,
                        pt[:, 512 * hh + nlo:512 * (hh + 1)],
                        start=(j == 0), stop=(j == jmax))
            # pO layout: A num rows 0-63 / den rows 64-127 (cols 0-511);
            # B den rows 0-63 / num rows 64-127 (cols 512-1023)
            osb = rp2.tile([128, 1024], F32, tag="osb", name="osb")
            nc.vector.tensor_copy(osb, pOp)
            recb = rp2.tile([128, 1024], F32, tag="recb", name="recb")
            nc.vector.reciprocal(recb[0:64, 0:512], osb[64:128, 0:512])
            nc.vector.reciprocal(recb[64:128, 512:1024],
                                 osb[0:64, 512:1024])
            nc.vector.tensor_mul(ypr[p][0:64, cs], osb[0:64, 0:512],
                                 recb[0:64, 0:512])
            nc.vector.tensor_mul(ypr[p][64:128, cs],
                                 osb[64:128, 512:1024],
                                 recb[64:128, 512:1024])

        def emit_oproj_chunk(c):
            cs = slice(c * 512, (c + 1) * 512)
            for dt in range(8):
                po = psp.tile([128, 512], F32, tag="ps", name="po")
                for kk in range(4):
                    nc.tensor.matmul(po,
                                     oc_sb[:, kk, dt * 128:(dt + 1) * 128],
                                     ypr[kk][:, cs],
                                     start=(kk == 0), stop=(kk == 3))
                osb2 = rp2.tile([128, 512], F32, tag="os", bufs=3,
                                name="osb2")
                _copy(nc, (dt + c) % 2 == 0, osb2, po)
                nc.gpsimd.dma_start(out=OT[dt * 128:(dt + 1) * 128, cs],
                                    in_=osb2)

        # emission: pair-0 projections, then pair-outer attention with
        # next-pair projections and V tiles interleaved
        for c in range(NC_):
            emit_proj_chunk(wq_sb, 0, c, qp[0])
        for c in range(NC_):
            emit_proj_chunk(wk_sb, 0, c, kp[0])
        for j in range(4):
            emit_v_tile(j)
        for p in range(NP):
            for c in range(NC_):
                emit_attn_chunk(p, c)
                if p == 0 and c < 3:
                    for j in range(4 * c + 4, 4 * c + 8):
                        emit_v_tile(j)
                if p < 3:
                    emit_proj_chunk(wq_sb, p + 1, c, qp[p + 1])
                    emit_proj_chunk(wk_sb, p + 1, c, kp[p + 1])
        # o_proj emitted last: lowest priority, so its matmuls fill PE
        # gaps while the final pair's exp-bound attention drains
        for c in range(NC_):
            emit_oproj_chunk(c)

    nc.finalize()
    return nc


def _prep_inputs(x, q_proj, k_proj, v_proj, o_proj):
    bf16 = ml_dtypes.bfloat16
    pos = np.arange(S, dtype=np.float64)
    inv = THETA ** (-np.arange(0, DK, 2, dtype=np.float64) / DK)   # [32]
    ang = inv[:, None] * pos[None, :]                              # [32, S]
    cos32, sin32 = np.cos(ang), np.sin(ang)
    # interleaved rope rows: row 2i -> freq i; sin sign: + on even rows
    # (multiplies e_i for the odd output), - on odd rows
    cos64 = np.repeat(cos32, 2, axis=0)                            # [64, S]
    sin64 = np.repeat(sin32, 2, axis=0)
    sin64[1::2] *= -1.0
    cos_big = np.tile(cos64, (2, 1)).astype(np.float32)            # [128, S]
    sin_big = np.tile(sin64, (2, 1)).astype(np.float32)

    in_maps = []
    for core in range(NCORES):
        b, g = core // 2, core % 2
        heads = [g * HPC + i for i in range(HPC)]
        nat = [h * DK + d_ for h in heads for d_ in range(DK)]

        def wlay(w):   # [D, HD] -> [128, NK, HD]
            return np.ascontiguousarray(
                w.reshape(NK, 128, HD).transpose(1, 0, 2)).astype(bf16)

        xt4 = x[b].T.reshape(NK, 128, NC_, 512).transpose(1, 2, 0, 3)
        oc4 = o_proj[:, nat].T.reshape(HD // 128, 128, D).transpose(1, 0, 2)
        in_maps.append({
            "XT": np.ascontiguousarray(xt4).astype(bf16),
            "WQ": wlay(q_proj[nat, :].T),
            "WK": wlay(k_proj[nat, :].T),
            "WV": wlay(v_proj[nat, :].T),
            "OC": np.ascontiguousarray(oc4).astype(bf16),
            "CC": cos_big,
            "SN": sin_big,
        })
    return in_maps


def _run(in_maps, **kw):
    if "nc" not in _CACHE:
        _CACHE["nc"] = _build_nc()
    return run_bass_kernel_spmd(_CACHE["nc"], in_maps,
                                core_ids=list(range(NCORES)), **kw)


def kernel(x, q_proj, k_proj, v_proj, o_proj):
    x = np.asarray(x, dtype=np.float32)
    in_maps = _prep_inputs(x,
                           np.asarray(q_proj, dtype=np.float32),
                           np.asarray(k_proj, dtype=np.float32),
                           np.asarray(v_proj, dtype=np.float32),
                           np.asarray(o_proj, dtype=np.float32))
    res = _run(in_maps)
    B = x.shape[0]
    out = np.empty((B, S, D), dtype=np.float32)
    for b in range(B):
        ot = res.results[2 * b]["OT"] + res.results[2 * b + 1]["OT"]
        out[b] = ot.T
    return out
